# revision 1
# baseline (speedup 1.0000x reference)
"""Windowed multi-head attention with relative-position bias on 8 TRN2 NeuronCores.

Data-parallel over batch: each of the 8 cores processes 4 of the 32 batch
elements end-to-end (QKV projection -> biased softmax attention -> output
projection); weights and the (precomputed, exponentiated) bias table are
replicated. No collectives are needed; the host shards inputs and
concatenates the per-core outputs.

Layout strategy (per core, b_local=4):
  - qkv is computed TRANSPOSED (q^T,k^T in [dh, n] layout) so that
    S^T = k^T.T @ q^T comes out in [j, i] layout with partition=j, which is
    exactly what the P@V matmul wants as its moving operand.
  - S^T matmuls have K=32, so 4 heads are packed per 128-partition tile and
    issued as bursts to 4 distinct PE row-groups (tile_position=(32h, 0)) ->
    they run concurrently on the systolic array (~3x measured on TRN2).
  - P@V matmuls have M=33 (32 outputs + a ones-column for the softmax
    denominator); heads alternate PE col-groups (tile_position=(0,0)/(0,64),
    each in its OWN psum bank) so neighbouring PVs overlap too.
  - softmax skips max-subtraction (scores are tiny: |s| < ~1) and the bias
    is applied as a multiplicative exp(bias) table on the vector engine.
  - the output projection consumes O^T = [d, n] directly and produces
    y^T = [d_out, n], which is already the required (b, d, H, W) layout.
  - matmul operands are bf16 (full PE rate); accumulation stays fp32.
"""

import numpy as np
import ml_dtypes

import concourse.bass as bass
import concourse.mybir as mybir
import concourse.tile as tile
from concourse import bacc
from concourse.bass_utils import run_bass_kernel_spmd

# problem shape (hardcoded; kernel.py must be self-contained)
B, D, WIN = 32, 256, 25
N = WIN * WIN            # 625 tokens
P = 128
NPAD = 640               # 5 j-tiles of 128
H, DH = 8, 32            # heads x head_dim
NCORES = 8
BL = B // NCORES         # 4 batch elements per core
KT_D = D // P            # 2 contraction tiles over d
NJT = NPAD // P          # 5 j-tiles
ICH = [(0, 320), (320, 305)]  # i-chunks (psum bank holds 512 fp32)
G = DH + 1               # 33: head group stride in V (32 outputs + ones col)
HP = 3                   # heads per 128-partition tile in O^T (3*33=99)
OPT = 3                  # O^T partition tiles (3+3+2 heads)
NSTR = 632               # padded i-stride (x2B = 4-byte aligned slices for DVE 2x)

F32 = mybir.dt.float32
BF16 = mybir.dt.bfloat16
EXP = mybir.ActivationFunctionType.Exp
NBF = ml_dtypes.bfloat16


def build_nc():
    # Bacc (not raw Bass): its compile pass legalizes multi-wait matmuls
    # (move_matmul_waits_to_ldweights / generate_event_semaphores), which
    # walrus codegen requires.
    nc = bacc.Bacc()
    x_d = nc.dram_tensor("x", (BL, D, N), BF16, kind="ExternalInput")
    wqk_d = nc.dram_tensor("w_qk", (D, 2 * D), BF16, kind="ExternalInput")
    wv_d = nc.dram_tensor("w_v", (D, D), BF16, kind="ExternalInput")
    wo_d = nc.dram_tensor("w_o", (OPT * P, D), BF16, kind="ExternalInput")
    eb_d = nc.dram_tensor("expb", (H, NPAD, N), BF16, kind="ExternalInput")
    out_d = nc.dram_tensor("out", (BL, D, N), F32, kind="ExternalOutput")

    with tile.TileContext(nc) as tc:
        with (
            tc.tile_pool(name="consts", bufs=1) as consts,
            tc.tile_pool(name="persist", bufs=1) as persist,
            tc.tile_pool(name="ebs", bufs=2) as ebs,
            tc.tile_pool(name="es", bufs=24) as es,
            tc.tile_pool(name="pvs", bufs=4) as pvs,
            tc.tile_pool(name="bcs", bufs=2) as bcs,
            tc.tile_pool(name="ys", bufs=2) as ys,
            tc.tile_pool(name="ps", bufs=2, space="PSUM") as ps,
        ):
            # ---------------- inputs (replicated weights, all-batch x) ----------------
            wqk = consts.tile([P, KT_D, 2 * D], BF16)
            nc.sync.dma_start(wqk[:], wqk_d.rearrange("(kt p) m -> p kt m", p=P))
            xall = consts.tile([P, BL, KT_D, N], BF16)
            nc.sync.dma_start(xall[:], x_d.rearrange("b (kt p) i -> p b kt i", p=P))
            wv = consts.tile([P, KT_D, D], BF16)
            nc.sync.dma_start(wv[:], wv_d.rearrange("(kt p) m -> p kt m", p=P))
            wo = consts.tile([P, OPT, D], BF16)
            nc.sync.dma_start(wo[:], wo_d.rearrange("(kt p) m -> p kt m", p=P))

            qkT, V, OT, den = {}, {}, {}, {}

            # ---------------- stage 1: projections, per batch element ----------------
            for b in range(BL):
                # q^T (free tiles 0,1 = W cols 0..255) and k^T (tiles 2,3),
                # 4 heads per 128-partition tile at offsets 0/32/64/96
                t_qkT = persist.tile([P, 4, NPAD], BF16, tag=f"qkT{b}", name=f"qkT{b}")
                nc.gpsimd.memset(t_qkT[:, 2:4, N:NPAD], 0.0)  # zero k^T j-pad
                for mt in range(4):
                    for c0, cw in ICH:
                        acc = ps.tile([P, 512], F32, tag="ps", name="acc", bufs=8)
                        for kt in range(KT_D):
                            nc.tensor.matmul(
                                acc[:, :cw],
                                wqk[:, kt, mt * P : (mt + 1) * P],
                                xall[:, b, kt, c0 : c0 + cw],
                                start=(kt == 0),
                                stop=(kt == KT_D - 1),
                            )
                        nc.vector.tensor_copy(out=t_qkT[:, mt, c0 : c0 + cw], in_=acc[:, :cw])

                # V in [j, head-grouped d] layout, with a ones column per head
                t_V = persist.tile([P, NJT, H * G], BF16, tag=f"V{b}", name=f"V{b}")
                nc.gpsimd.memset(t_V[:], 1.0)
                for jt in range(NJT):
                    jr = min(P, N - jt * P)  # 128,128,128,128,113
                    acc = ps.tile([P, 512], F32, tag="ps", name="acc", bufs=8)
                    for kt in range(KT_D):
                        nc.tensor.matmul(
                            acc[:jr, :D],
                            xall[:, b, kt, jt * P : jt * P + jr],
                            wv[:, kt, :],
                            start=(kt == 0),
                            stop=(kt == KT_D - 1),
                        )
                    nc.vector.tensor_copy(
                        out=t_V[:jr, jt].rearrange("p (h g) -> p h g", g=G)[:, :, :DH],
                        in_=acc[:jr, :D].rearrange("p (h g) -> p h g", g=DH),
                    )

                # O^T staging: head h lives at rows (h%3)*33.. of ptile h//3
                t_OT = persist.tile([P, OPT, NSTR], BF16, tag=f"OT{b}", name=f"OT{b}")
                nc.gpsimd.memset(t_OT[:], 0.0)  # pad rows must be 0, not NaN garbage
                t_den = persist.tile([H, N], BF16, tag=f"den{b}", name=f"den{b}")
                qkT[b], V[b], OT[b], den[b] = t_qkT, t_V, t_OT, t_den

            # ---------------- stage 2: attention ----------------
            # head groups of 4 (one q/k partition tile); per (group, batch,
            # i-chunk): S -> exp -> bias-mult per (j-tile, head), then P@V
            # accumulation, all rotating through one 8-bank psum pool.
            ebg = {}
            for g in range(H // 4):
                ebg[g] = ebs.tile([P, 4, NJT, NSTR], BF16, tag="ebg", name=f"ebg{g}")
                nc.sync.dma_start(
                    ebg[g][:, :, :, :N],
                    eb_d[4 * g : 4 * (g + 1)].rearrange("h (jt p) i -> p h jt i", p=P),
                )
            for b in range(BL):
                for g in range(H // 4):
                    pvSg = pvs.tile([G, 4, NSTR], BF16, tag="pvS", name=f"pvS{g}{b}")
                    for ci, (c0, cw) in enumerate(ICH):
                        E2 = {}
                        for jt in range(NJT):
                            # 4 S matmuls issued adjacently -> 4 PE row-groups
                            # stream concurrently; exp/bias-mult trail as groups
                            st4 = []
                            for hq in range(4):
                                off = hq * DH
                                st = ps.tile([P, 512], F32, tag="ps", name="st", bufs=8)
                                nc.tensor.matmul(
                                    st[:, :cw],
                                    qkT[b][off : off + DH, 2 + g, jt * P : (jt + 1) * P],
                                    qkT[b][off : off + DH, g, c0 : c0 + cw],
                                    tile_position=(off, 0),
                                )
                                st4.append(st)
                            for hq in range(4):
                                E = es.tile([P, 512], BF16, tag="E", name=f"E{hq}", bufs=12)
                                nc.scalar.activation(E[:, :cw], st4[hq][:, :cw], EXP)
                                E2[jt, hq] = E
                            for hq in range(4):
                                Eo = es.tile([P, 512], BF16, tag="Eo", name=f"Eo{hq}", bufs=12)
                                nc.vector.tensor_mul(
                                    out=Eo[:, :cw],
                                    in0=E2[jt, hq][:, :cw],
                                    in1=ebg[g][:, hq, jt, c0 : c0 + cw],
                                )
                                E2[jt, hq] = Eo
                        pvt = [
                            ps.tile([P, 512], F32, tag="ps", name=f"pv{hq}", bufs=8)
                            for hq in range(4)
                        ]
                        for jt in range(NJT):
                            for hq in range(4):
                                h = 4 * g + hq
                                row = (hq % 2) * 64
                                nc.tensor.matmul(
                                    pvt[hq][row : row + G, :cw],
                                    V[b][:, jt, h * G : (h + 1) * G],
                                    E2[jt, hq][:, :cw],
                                    start=(jt == 0),
                                    stop=(jt == NJT - 1),
                                    tile_position=(0, row),
                                )
                        for hq in range(4):
                            row = (hq % 2) * 64
                            nc.vector.tensor_copy(
                                out=pvSg[:, hq, c0 : c0 + cw],
                                in_=pvt[hq][row : row + G, :cw],
                            )
                    for hq in range(4):
                        h = 4 * g + hq
                        pt, slot = divmod(h, HP)
                        nc.sync.dma_start(
                            OT[b][slot * G : slot * G + DH, pt, :N], pvSg[:DH, hq, :N]
                        )
                    nc.sync.dma_start(
                        den[b][4 * g : 4 * (g + 1), :], pvSg[DH : DH + 1, :, :N]
                    )
            # stage 3 trails the whole program: lowest scheduler priority, so
            # its ops backfill engine gaps instead of preempting attention
            for b in range(BL):
                _stage3(nc, b, OT, den, wo, bcs, ys, ps, out_d)

    return nc


def _stage3(nc, b, OT, den, wo, bcs, ys, ps, out_d):
    """Normalize O^T by the softmax denominators and apply W_out."""
    denf = bcs.tile([H, N], F32, tag="denf", name=f"denf{b}")
    nc.vector.tensor_copy(out=denf[:], in_=den[b][:])
    scr = bcs.tile([H, N], F32, tag="scr", name=f"scr{b}")
    nc.vector.reciprocal_approx_accurate(out=denf[:], in_=denf[:], scratch=scr[:])
    denb = bcs.tile([H, NSTR], BF16, tag="denb", name=f"denb{b}")
    nc.vector.tensor_copy(out=denb[:, :N], in_=denf[:])
    bc = bcs.tile([P, OPT, NSTR], BF16, tag="bc", name=f"bc{b}")
    for pt in range(OPT):
        nh = min(HP, H - pt * HP)  # 3,3,2
        nc.sync.dma_start(
            bc[: nh * G, pt, :N],
            denb[pt * HP : pt * HP + nh, None, :N].to_broadcast((nh, G, N)),
        )
    for pt in range(OPT):
        nh = min(HP, H - pt * HP)
        for c0, cw in ICH:
            nc.vector.tensor_mul(
                out=OT[b][: nh * G, pt, c0 : c0 + cw],
                in0=OT[b][: nh * G, pt, c0 : c0 + cw],
                in1=bc[: nh * G, pt, c0 : c0 + cw],
            )
    yb = ys.tile([P, KT_D, N], F32, tag="yb", name=f"yb{b}")
    for mt in range(KT_D):
        for c0, cw in ICH:
            yp = ps.tile([P, 512], F32, tag="ps", name="yp", bufs=8)
            for kt in range(OPT):
                nc.tensor.matmul(
                    yp[:, :cw],
                    wo[:, kt, mt * P : (mt + 1) * P],
                    OT[b][:, kt, c0 : c0 + cw],
                    start=(kt == 0),
                    stop=(kt == OPT - 1),
                )
            nc.scalar.copy(yb[:, mt, c0 : c0 + cw], yp[:, :cw])
    nc.sync.dma_start(out_d[b].rearrange("(mt p) i -> p mt i", p=P), yb[:])


def _host_prep(W_qkv, W_out, rel_emb):
    scale = DH ** -0.5
    wqk = np.ascontiguousarray(W_qkv[:, : 2 * D]).copy()
    wqk[:, :D] *= scale  # fold q scaling into the weights
    wv = np.ascontiguousarray(W_qkv[:, 2 * D :])
    # W_out rows rearranged into the packed O^T layout; denominator/pad rows zero
    wo = np.zeros((OPT * P, D), np.float32)
    for h in range(H):
        pt, slot = divmod(h, HP)
        wo[pt * P + slot * G : pt * P + slot * G + DH] = W_out[h * DH : (h + 1) * DH]
    # relative-position bias -> exp(bias)^T, padded along j to 640 with zeros
    pos = np.arange(WIN)
    gi, gj = np.meshgrid(pos, pos, indexing="ij")
    grid = np.stack([gi.reshape(-1), gj.reshape(-1)], -1)
    rel = grid[:, None, :] - grid[None, :, :] + (WIN - 1)
    idx = rel[..., 0] * (2 * WIN - 1) + rel[..., 1]  # [i, j]
    eb = np.zeros((H, NPAD, N), np.float32)
    eb[:, :N, :] = np.exp(rel_emb[idx]).transpose(2, 1, 0)  # -> [h, j, i]
    return wqk.astype(NBF), wv.astype(NBF), wo.astype(NBF), eb.astype(NBF)


def _install_ntff_hook():
    """This image lacks antenv.axon_hooks; shim it and register the ctypes
    NTFF profiling hook so trace=True yields exec_time_ns. Bench-only."""
    import sys
    import types

    if "antenv.axon_hooks" not in sys.modules:
        mod = types.ModuleType("antenv.axon_hooks")
        mod._hook = None
        mod.set_axon_ntff_profile_hook = lambda h: setattr(mod, "_hook", h)
        mod.get_axon_ntff_profile_hook = lambda: mod._hook
        sys.modules["antenv.axon_hooks"] = mod
    try:
        from trn_agent_boot.trn_boot import _ntff_profile_via_ctypes

        hook = _ntff_profile_via_ctypes("/opt/axon/libaxon_pjrt.so")
        sys.modules["antenv.axon_hooks"].set_axon_ntff_profile_hook(hook)
    except Exception as e:  # degrade to untimed run
        print(f"NTFF hook install failed ({e}); running without trace")


def kernel(x, W_qkv, W_out, rel_emb, _bench=False):
    x = np.ascontiguousarray(
        np.asarray(x, np.float32).reshape(B, D, N).astype(NBF)
    )
    wqk, wv, wo, eb = _host_prep(
        np.asarray(W_qkv, np.float32),
        np.asarray(W_out, np.float32),
        np.asarray(rel_emb, np.float32),
    )
    nc = build_nc()
    nc.finalize()
    in_maps = [
        {"x": x[c * BL : (c + 1) * BL], "w_qk": wqk, "w_v": wv, "w_o": wo, "expb": eb}
        for c in range(NCORES)
    ]
    if _bench:
        _install_ntff_hook()
    res = run_bass_kernel_spmd(nc, in_maps, core_ids=list(range(NCORES)), trace=_bench)
    if _bench:
        kernel._last = res
    out = np.concatenate([np.asarray(res.results[c]["out"]) for c in range(NCORES)], axis=0)
    return out.reshape(B, D, WIN, WIN).astype(np.float32)



# revision 38
# speedup vs baseline: 1.3499x; 1.3499x over previous
"""Windowed multi-head attention with relative-position bias on 8 TRN2 NeuronCores.

Data-parallel over batch: each of the 8 cores processes 4 of the 32 batch
elements end-to-end; weights and the exponentiated bias table are replicated.

v2 layout strategy (per core, b_local=4), derived from trace analysis of v1:
the Activation engine (exp over 12.8M scores/core) was the bottleneck at ~59
G elem/s because each ACTIVATE carried ~425ns of fixed overhead on [128,320]
tiles. This version restructures stage 2 around few, huge activations:

  - S^T psum is laid out as a 4-bank tile (4 heads x 512 i-cols) plus a
    1-bank tile (4 heads x 113 i-cols packed), so softmax runs as TWO
    activations per (batch, head-group, j-tile): [128,2048] + [128,452].
  - The S(jt+1)-after-exp(jt) psum reuse chain is the pipeline clock:
    cycle = matmul(S) + exp = ~3us, with the ACT engine ~100% busy.
  - bias multiply stays on DVE as two big tensor_mul's per j-tile.
  - P@V pairs share psum banks (rows 0:33 / 64:97) -> 3 PV banks per unit;
    psum = 4(S) + 1(Sc1) + 3(PV) = 8 banks exactly.
  - PV outputs are copied psum->sbuf as 3 bf16 casts, then rearranged into
    the head-major O^T layout by Pool-queue DMAs (cheap dispatch).
  - O^T uses a clean (head,dh)-major 256-row layout (den kept separately),
    so W_out needs no repacking and the out-projection is 2 exact K-tiles.
  - normalize-and-cast is fused into one tensor_mul per batch; output is
    written bf16 (halves the output DMA).
  - stage-1 projections of batch b+1 and stage-3 of batch b are emitted
    between attention units as tensor-engine filler.
"""

import numpy as np
import ml_dtypes

import concourse.bass as bass
import concourse.mybir as mybir
import concourse.tile as tile
from concourse import bacc
from concourse.bass_utils import run_bass_kernel_spmd

# problem shape (hardcoded; kernel.py must be self-contained)
B, D, WIN = 32, 256, 25
N = WIN * WIN            # 625 tokens
P = 128
NPAD = 640               # 5 j-tiles of 128
H, DH = 8, 32            # heads x head_dim
NCORES = 8
BL = B // NCORES         # 4 batch elements per core
KT_D = D // P            # 2 contraction tiles over d
NJT = NPAD // P          # 5 j-tiles
C0, C1 = 512, 113        # i-chunks (chunk0 = one psum bank per head)
ICH = [(0, C0), (C0, C1)]
G = DH + 1               # 33: PV output rows per head (32 outputs + den)
NSTR = 632               # padded i-stride (even # of bf16 for DVE 2x slices)

F32 = mybir.dt.float32
BF16 = mybir.dt.bfloat16
EXP = mybir.ActivationFunctionType.Exp
NBF = ml_dtypes.bfloat16
ACT_SPAN_BANKS = True  # one [128,2048] exp per j-tile vs 4 per-bank exps


def _stage1(nc, b, wqk, wv, xall, persist, ps):
    """QKV projections for batch b. Returns (tiles, group-closures): the
    closures each emit one small psum group through the "sc1" bank and can be
    interleaved into an attention unit's pipeline cycles as PE filler."""
    t_qkT = persist.tile([P, 4, NPAD], BF16, tag=f"qkT{b}", name=f"qkT{b}")
    nc.gpsimd.memset(t_qkT[:, 2:4, N:NPAD], 0.0)  # zero k^T j-pad
    t_V = persist.tile([P, NJT, H, G], BF16, tag=f"V{b}", name=f"V{b}")
    nc.gpsimd.memset(t_V[:], 1.0)
    t_q1p = persist.tile([P, 2, 4, C1], BF16, tag=f"q1p{b}", name=f"q1p{b}")
    nc.gpsimd.memset(t_q1p[:], 0.0)
    groups = []

    def qk_c0(mt):
        def emit():
            acc = ps.tile([P, 512], F32, tag="pv", name="acc", bufs=3)
            for kt in range(KT_D):
                nc.tensor.matmul(
                    acc[:],
                    wqk[:, kt, mt * P : (mt + 1) * P],
                    xall[:, b, kt, 0:C0],
                    start=(kt == 0),
                    stop=(kt == KT_D - 1),
                )
            nc.scalar.copy(t_qkT[:, mt, 0:C0], acc[:])
        return emit

    def qk_c1():
        acc1 = ps.tile([P, 4 * C1], F32, tag="sc1", name="acc1", bufs=1)
        for mt in range(4):
            for kt in range(KT_D):
                nc.tensor.matmul(
                    acc1[:, mt * C1 : (mt + 1) * C1],
                    wqk[:, kt, mt * P : (mt + 1) * P],
                    xall[:, b, kt, C0:N],
                    start=(kt == 0),
                    stop=(kt == KT_D - 1),
                    skip_group_check=True,
                )
        nc.vector.tensor_copy(
            out=t_qkT[:, :, C0:N],
            in_=acc1[:].rearrange("p (mt c) -> p mt c", c=C1),
        )
        for g in range(2):
            for hq in range(4):
                off = hq * DH
                nc.vector.tensor_copy(
                    out=t_q1p[off : off + DH, g, hq, :],
                    in_=t_qkT[off : off + DH, g, C0:N],
                )

    def v_proj(jt):
        def emit():
            jr = min(P, N - jt * P)
            acc = ps.tile([P, 512], F32, tag="pv", name="accv", bufs=3)
            for kt in range(KT_D):
                nc.tensor.matmul(
                    acc[:jr, :],
                    xall[:, b, kt, jt * P : jt * P + jr],
                    wv[:, kt, :],
                    start=(kt == 0),
                    stop=(kt == KT_D - 1),
                )
            nc.vector.tensor_copy(
                out=t_V[:jr, jt, :, :DH],
                in_=acc[:jr, :D].rearrange("p (h g) -> p h g", g=DH),
            )
        return emit

    # qk_c1 copies q1p for BOTH g from the c1 columns written by qk_c1 itself,
    # so it must run after... it IS the writer of those columns. Order: c0
    # tiles first (they fill cols 0:512 and q1p c0?? no - q1p holds c1 cols),
    # then qk_c1.
    for mt in range(4):
        groups.append(qk_c0(mt))
    groups.append(qk_c1)
    for jt in range(NJT):
        groups.append(v_proj(jt))
    return (t_qkT, t_V, t_q1p), groups


def _unit(nc, b, g, qkT, V, q1pad, ebg0, ebg1, OT, den, es, pvs, ps, fillers=None):
    """Attention for (batch b, head-group g): 4 heads, softmax, P@V."""
    import os

    um = int(os.environ.get("UPARTS", "31"))  # 1=Smm 2=acts 4=muls 8=pv 16=out
    # PV accumulators: zeroed up-front, accumulated into WITHOUT start flags
    # (multiple head-groups share each bank; start=True would zero the whole
    # 2KB region under the neighbours).
    pv0 = ps.tile([P, 512], F32, tag="pv", name="pv0", bufs=3)
    pv1 = ps.tile([P, 512], F32, tag="pv", name="pv1", bufs=3)
    pv2 = ps.tile([P, 512], F32, tag="pv", name="pv2", bufs=3)
    nc.vector.memset(pv0[:], 0.0)
    nc.vector.memset(pv1[:], 0.0)
    nc.vector.memset(pv2[:], 0.0)
    pvp = [pv0, pv1, pv2]
    q1p = q1pad[b][:, g]
    # Software pipeline: the act0(jt) -> S0-matmuls(jt+1) psum-reuse chain is
    # the clock (~2.9us/jt). Everything else trails behind it: act1(jt-1)
    # runs BEFORE act0(jt) so the SC1 bank frees early, mul1 runs on the
    # idle GpSimd, and P@V of chunk1 trails TWO j-tiles so its operands are
    # always ready when the in-order PE queue reaches it.
    def pv_c0(jt, Eo0):
        for hq in range(4):
            h = 4 * g + hq
            row = (hq % 2) * 64
            nc.tensor.matmul(
                pvp[hq // 2][row : row + G, 0:C0],
                V[b][:, jt, h, :],
                Eo0[:, hq * C0 : (hq + 1) * C0],
                start=False,
                stop=False,
                tile_position=(0, row),
                skip_group_check=True,
            )

    def pv_c1(jt, Eo1):
        for hq in range(4):
            h = 4 * g + hq
            row = (hq % 2) * 64
            nc.tensor.matmul(
                pvp[2][row : row + G, (hq // 2) * 128 : (hq // 2) * 128 + C1],
                V[b][:, jt, h, :],
                Eo1[:, hq * C1 : (hq + 1) * C1],
                start=False,
                stop=False,
                tile_position=(0, row),
                skip_group_check=True,
            )

    p1 = None  # (jt, Eo0, s1) trailing one step
    p2 = None  # (jt, Eo1) trailing two steps
    for jt in range(NJT):
        # S0 split in two 2-bank halves so the exp of half A overlaps the
        # S-matmuls of half B of the NEXT j-tile (shorter psum-reuse chain)
        s0a = ps.tile([P, 2 * C0], F32, tag="s0a", name="s0a", bufs=1)
        for hq in range(2):
            off = hq * DH
            nc.tensor.matmul(
                s0a[:, hq * C0 : (hq + 1) * C0],
                qkT[b][off : off + DH, 2 + g, jt * P : (jt + 1) * P],
                qkT[b][off : off + DH, g, 0:C0],
                tile_position=(off, 0),
            )
        if p1 is not None:
            E1 = es.tile([P, 4 * C1], BF16, tag="E1", name="E1", bufs=3)
            nc.scalar.activation(E1[:], p1[2][:], EXP)
        E0a = es.tile([P, 2 * C0], BF16, tag="E0a", name="E0a", bufs=3)
        nc.scalar.activation(E0a[:], s0a[:], EXP)
        s0b = ps.tile([P, 2 * C0], F32, tag="s0b", name="s0b", bufs=1)
        for hq in range(2, 4):
            off = hq * DH
            nc.tensor.matmul(
                s0b[:, (hq - 2) * C0 : (hq - 1) * C0],
                qkT[b][off : off + DH, 2 + g, jt * P : (jt + 1) * P],
                qkT[b][off : off + DH, g, 0:C0],
                tile_position=(off, 0),
            )
        E0b = es.tile([P, 2 * C0], BF16, tag="E0b", name="E0b", bufs=3)
        nc.scalar.activation(E0b[:], s0b[:], EXP)
        Eo0 = es.tile([P, 4 * C0], BF16, tag="Eo0", name="Eo0", bufs=3)
        nc.vector.tensor_mul(
            out=Eo0[:, 0 : 2 * C0], in0=E0a[:], in1=ebg0[g][:, jt, 0 : 2 * C0]
        )
        nc.vector.tensor_mul(
            out=Eo0[:, 2 * C0 :], in0=E0b[:], in1=ebg0[g][:, jt, 2 * C0 :]
        )
        if p1 is not None:
            Eo1 = es.tile([P, 4 * C1], BF16, tag="Eo1", name="Eo1", bufs=3)
            nc.gpsimd.tensor_mul(out=Eo1[:], in0=E1[:], in1=ebg1[g][:, p1[0], :])
            pv_c0(p1[0], p1[1])
            if p2 is not None:
                pv_c1(p2[0], p2[1])
            p2 = (p1[0], Eo1)
        if fillers:
            fillers.pop(0)()
        s1 = ps.tile([P, 4 * C1], F32, tag="sc1", name="s1", bufs=1)
        for hq in range(4):
            nc.tensor.matmul(
                s1[:, hq * C1 : (hq + 1) * C1],
                qkT[b][:, 2 + g, jt * P : (jt + 1) * P],
                q1p[:, hq, :],
                skip_group_check=True,
            )
        p1 = (jt, Eo0, s1)
    # drain the pipeline tail
    E1 = es.tile([P, 4 * C1], BF16, tag="E1", name="E1", bufs=3)
    nc.scalar.activation(E1[:], p1[2][:], EXP)
    Eo1 = es.tile([P, 4 * C1], BF16, tag="Eo1", name="Eo1", bufs=3)
    nc.gpsimd.tensor_mul(out=Eo1[:], in0=E1[:], in1=ebg1[g][:, p1[0], :])
    pv_c0(p1[0], p1[1])
    pv_c1(p2[0], p2[1])
    pv_c1(p1[0], Eo1)
    if fillers:
        while fillers:
            fillers.pop(0)()

    # --- drain PV psum -> bf16 staging -> O^T/den assembly DMAs ---
    pvS = pvs.tile([P, 1280], BF16, tag="pvS", name="pvS", bufs=3)
    nc.vector.tensor_copy(out=pvS[:, 0:512], in_=pv0[:])
    nc.vector.tensor_copy(out=pvS[:, 512:1024], in_=pv1[:])
    nc.vector.tensor_copy(out=pvS[:, 1024:1265], in_=pv2[:, 0:241])
    # heads hq = 2*colblock + partblock; dh rows 0:32 / 64:96, den at 32/96
    for cb in range(2):
        for pb in range(2):
            hq = 2 * cb + pb
            h0 = 32 * g + hq
            nc.sync.dma_start(
                OT[b][hq * DH : (hq + 1) * DH, g, 0:C0],
                pvS[pb * 64 : pb * 64 + DH, cb * 512 : (cb + 1) * 512],
            )
            nc.sync.dma_start(
                den[b][h0 : h0 + 1, 0:C0],
                pvS[pb * 64 + DH : pb * 64 + DH + 1, cb * 512 : (cb + 1) * 512],
            )
            # chunk1: packed at 128-col blocks inside pv2's staging area
            c1b = 1024 + cb * 128
            nc.sync.dma_start(
                OT[b][hq * DH : (hq + 1) * DH, g, C0:N],
                pvS[pb * 64 : pb * 64 + DH, c1b : c1b + C1],
            )
            nc.sync.dma_start(
                den[b][h0 : h0 + 1, C0:N],
                pvS[pb * 64 + DH : pb * 64 + DH + 1, c1b : c1b + C1],
            )


def _pv_matmuls(nc, b, g, V, pvp, eo, jt):
    """P@V for one j-tile; pairs of heads share a psum bank (rows 0/64).
    Accumulates onto the memset zeros — no start/stop flags."""
    Eo0, Eo1 = eo
    for hq in range(4):
        h = 4 * g + hq
        row = (hq % 2) * 64
        nc.tensor.matmul(
            pvp[hq // 2][row : row + G, 0:C0],
            V[b][:, jt, h, :],
            Eo0[:, hq * C0 : (hq + 1) * C0],
            start=False,
            stop=False,
            tile_position=(0, row),
            skip_group_check=True,
        )
    for hq in range(4):
        h = 4 * g + hq
        row = (hq % 2) * 64
        nc.tensor.matmul(
            pvp[2][row : row + G, (hq // 2) * 128 : (hq // 2) * 128 + C1],
            V[b][:, jt, h, :],
            Eo1[:, hq * C1 : (hq + 1) * C1],
            start=False,
            stop=False,
            tile_position=(0, row),
            skip_group_check=True,
        )


def _stage3(nc, b, OT, den, wo, bco, bcs, ys, ps, out_d):
    """Normalize O^T by softmax denominators (broadcast built ON THE PE:
    ones-pattern stationary x rden moving), apply W_out, store bf16."""
    # den rows live at partitions kt*32 + hq so the broadcast matmuls get
    # legal tile positions; pad rows are memset to 1.0 (recip-safe)
    denf = bcs.tile([64, N], F32, tag="denf", name="denf", bufs=2)
    nc.vector.tensor_copy(out=denf[:], in_=den[b][:, :N])
    scr = bcs.tile([64, N], F32, tag="scr", name="scr", bufs=2)
    rden = bcs.tile([64, N], F32, tag="rden", name="rden", bufs=2)
    nc.vector.reciprocal_approx_accurate(out=rden[:], in_=denf[:], scratch=scr[:])
    rdenb = bcs.tile([64, NSTR], BF16, tag="rdenb", name="rdenb", bufs=2)
    nc.vector.tensor_copy(out=rdenb[:, :N], in_=rden[:])
    bcp = {}
    for kt in range(KT_D):
        bcp[kt] = ps.tile([P, 512], F32, tag="pv", name=f"bcp{kt}", bufs=3)
        nc.tensor.matmul(
            bcp[kt][:],
            bco[kt * 32 : kt * 32 + 4, :],
            rdenb[kt * 32 : kt * 32 + 4, 0:C0],
            tile_position=(kt * 32, 0),
        )
    for kt in range(KT_D):
        # separate sc1 allocations: two row-group streams must not share a
        # psum bank concurrently; the WAR chain via the tag serializes them
        bcc = ps.tile([P, 4 * C1], F32, tag="sc1", name=f"bcc{kt}", bufs=1)
        nc.tensor.matmul(
            bcc[:, 0:C1],
            bco[kt * 32 : kt * 32 + 4, :],
            rdenb[kt * 32 : kt * 32 + 4, C0:N],
            tile_position=(kt * 32, 0),
            skip_group_check=True,
        )
        nc.vector.tensor_mul(
            out=OT[b][:, kt, 0:C0], in0=OT[b][:, kt, 0:C0], in1=bcp[kt][:]
        )
        nc.vector.tensor_mul(
            out=OT[b][:, kt, C0:N], in0=OT[b][:, kt, C0:N], in1=bcc[:, 0:C1]
        )
    yb = ys.tile([P, KT_D, NSTR], BF16, tag="yb", name="yb", bufs=2)
    for mt in range(KT_D):
        yp = ps.tile([P, 512], F32, tag="pv", name="yp", bufs=3)
        for kt in range(KT_D):
            nc.tensor.matmul(
                yp[:],
                wo[:, kt, mt * P : (mt + 1) * P],
                OT[b][:, kt, 0:C0],
                start=(kt == 0),
                stop=(kt == KT_D - 1),
            )
        nc.scalar.copy(yb[:, mt, 0:C0], yp[:])
    # ragged i-chunk: both mt tiles in the 1-bank "sc1" slot
    yp1 = ps.tile([P, 4 * C1], F32, tag="sc1", name="yp1", bufs=1)
    for mt in range(KT_D):
        for kt in range(KT_D):
            nc.tensor.matmul(
                yp1[:, mt * C1 : (mt + 1) * C1],
                wo[:, kt, mt * P : (mt + 1) * P],
                OT[b][:, kt, C0:N],
                start=(kt == 0),
                stop=(kt == KT_D - 1),
                skip_group_check=True,
            )
    nc.vector.tensor_copy(
        out=yb[:, :, C0:N],
        in_=yp1[:, : 2 * C1].rearrange("p (mt c) -> p mt c", c=C1),
    )
    for mt in range(KT_D):
        nc.sync.dma_start(
            out_d[b][mt * P : (mt + 1) * P, :], yb[:, mt, :N]
        )


def build_nc():
    nc = bacc.Bacc()
    x_d = nc.dram_tensor("x", (P, BL, KT_D, N), BF16, kind="ExternalInput")
    wqk_d = nc.dram_tensor("w_qk", (D, 2 * D), BF16, kind="ExternalInput")
    wv_d = nc.dram_tensor("w_v", (D, 2 * D), BF16, kind="ExternalInput")
    wo_d = nc.dram_tensor("w_o", (D, D), BF16, kind="ExternalInput")
    eb0_d = nc.dram_tensor("expb0", (2, NJT, P, 4 * C0), BF16, kind="ExternalInput")
    eb1_d = nc.dram_tensor("expb1", (2, NJT, P, 4 * C1), BF16, kind="ExternalInput")
    bco_d = nc.dram_tensor("bcones", (64, P), BF16, kind="ExternalInput")
    out_d = nc.dram_tensor("out", (BL, D, N), BF16, kind="ExternalOutput")

    with tile.TileContext(nc) as tc:
        with (
            tc.tile_pool(name="consts", bufs=1) as consts,
            tc.tile_pool(name="persist", bufs=1) as persist,
            tc.tile_pool(name="ebs", bufs=1) as ebs,
            tc.tile_pool(name="es", bufs=2) as es,
            tc.tile_pool(name="pvs", bufs=2) as pvs,
            tc.tile_pool(name="bcs", bufs=2) as bcs,
            tc.tile_pool(name="ys", bufs=2) as ys,
            tc.tile_pool(name="ps", bufs=1, space="PSUM") as ps,
        ):
            # ---------------- inputs ----------------
            wqk = consts.tile([P, KT_D, 2 * D], BF16)
            nc.sync.dma_start(wqk[:], wqk_d.rearrange("(kt p) m -> p kt m", p=P))
            xall = consts.tile([P, BL, KT_D, N], BF16)
            nc.sync.dma_start(xall[:], x_d[:, :, :, :])
            wv = consts.tile([P, KT_D, 2 * D], BF16)
            nc.sync.dma_start(wv[:], wv_d.rearrange("(kt p) m -> p kt m", p=P))
            wo = consts.tile([P, KT_D, D], BF16)
            nc.sync.dma_start(wo[:], wo_d.rearrange("(kt p) m -> p kt m", p=P))
            bco = consts.tile([64, P], BF16)
            nc.sync.dma_start(bco[:], bco_d[:, :])
            ebg0, ebg1 = {}, {}
            for g in range(2):
                ebg0[g] = ebs.tile(
                    [P, NJT, 4 * C0], BF16, tag=f"eb0g{g}", name=f"eb0g{g}", bufs=1
                )
                nc.sync.dma_start(ebg0[g][:], eb0_d[g].rearrange("jt p c -> p jt c"))
                ebg1[g] = ebs.tile(
                    [P, NJT, 4 * C1], BF16, tag=f"eb1g{g}", name=f"eb1g{g}", bufs=1
                )
                nc.sync.dma_start(ebg1[g][:], eb1_d[g].rearrange("jt p c -> p jt c"))

            # per-batch persistent tiles
            qkT, V, OT, den, q1pad = {}, {}, {}, {}, {}
            for b in range(BL):
                OT[b] = persist.tile(
                    [P, KT_D, NSTR], BF16, tag=f"OT{b}", name=f"OT{b}"
                )
                den[b] = persist.tile(
                    [64, NSTR], BF16, tag=f"den{b}", name=f"den{b}"
                )
                nc.gpsimd.memset(den[b][:], 1.0)

            # stage 1 for b=0 up front; later batches interleave as PE filler
            import os

            lvl = os.environ.get("KBISECT", "full")
            (qkT[0], V[0], q1pad[0]), g0 = _stage1(nc, 0, wqk, wv, xall, persist, ps)
            for grp in g0:
                grp()
            if lvl == "s1":
                pass
            else:
                for b in range(BL):
                    _unit(nc, b, 0, qkT, V, q1pad, ebg0, ebg1, OT, den, es, pvs, ps)
                    if b + 1 < BL:
                        (qkT[b + 1], V[b + 1], q1pad[b + 1]), grps = _stage1(
                            nc, b + 1, wqk, wv, xall, persist, ps
                        )
                        for grp in grps:
                            grp()
                    if b > 0:
                        _stage3(nc, b - 1, OT, den, wo, bco, bcs, ys, ps, out_d)
                    _unit(nc, b, 1, qkT, V, q1pad, ebg0, ebg1, OT, den, es, pvs, ps)
                _stage3(nc, BL - 1, OT, den, wo, bco, bcs, ys, ps, out_d)
    return nc


def _host_prep(W_qkv, W_out, rel_emb):
    scale = DH ** -0.5
    wqk = np.ascontiguousarray(W_qkv[:, : 2 * D]).copy()
    wqk[:, :D] *= scale  # fold q scaling into the weights
    wv = np.zeros((D, 2 * D), np.float32)
    wv[:, :D] = W_qkv[:, 2 * D :]
    wo = np.ascontiguousarray(W_out)  # (h,dh)-major rows match O^T layout
    # relative-position bias -> exp(bias)^T in the packed S-psum layouts
    pos = np.arange(WIN)
    gi, gj = np.meshgrid(pos, pos, indexing="ij")
    grid = np.stack([gi.reshape(-1), gj.reshape(-1)], -1)
    rel = grid[:, None, :] - grid[None, :, :] + (WIN - 1)
    idx = rel[..., 0] * (2 * WIN - 1) + rel[..., 1]  # [i, j]
    ebT = np.exp(rel_emb[idx]).transpose(2, 1, 0)  # -> [h, j, i]
    ebp = np.zeros((H, NPAD, N), np.float32)
    ebp[:, :N, :] = ebT
    e = ebp.reshape(2, 4, NJT, P, N)  # [g, hq, jt, p, i]
    eb0 = np.ascontiguousarray(
        e[:, :, :, :, :C0].transpose(0, 2, 3, 1, 4).reshape(2, NJT, P, 4 * C0)
    )
    eb1 = np.ascontiguousarray(
        e[:, :, :, :, C0:].transpose(0, 2, 3, 1, 4).reshape(2, NJT, P, 4 * C1)
    )
    bco = np.zeros((64, P), np.float32)
    for kt in range(KT_D):
        for j in range(4):
            bco[kt * 32 + j, j * DH : (j + 1) * DH] = 1.0
    return (
        wqk.astype(NBF),
        wv.astype(NBF),
        wo.astype(NBF),
        eb0.astype(NBF),
        eb1.astype(NBF),
        bco.astype(NBF),
    )


def _install_ntff_hook():
    """This image lacks antenv.axon_hooks; shim it and register the ctypes
    NTFF profiling hook so trace=True yields exec_time_ns. Bench-only."""
    import sys
    import types

    if "antenv.axon_hooks" not in sys.modules:
        mod = types.ModuleType("antenv.axon_hooks")
        mod._hook = None
        mod.set_axon_ntff_profile_hook = lambda h: setattr(mod, "_hook", h)
        mod.get_axon_ntff_profile_hook = lambda: mod._hook
        sys.modules["antenv.axon_hooks"] = mod
    try:
        from trn_agent_boot.trn_boot import _ntff_profile_via_ctypes

        hook = _ntff_profile_via_ctypes("/opt/axon/libaxon_pjrt.so")
        sys.modules["antenv.axon_hooks"].set_axon_ntff_profile_hook(hook)
    except Exception as e:  # degrade to untimed run
        print(f"NTFF hook install failed ({e}); running without trace")


def kernel(x, W_qkv, W_out, rel_emb, _bench=False):
    # pre-transpose per-core x to the sbuf layout so the load DMA is one
    # contiguous 10KB descriptor per partition instead of 1024 row gathers
    x = np.asarray(x, np.float32).reshape(NCORES, BL, KT_D, P, N)
    x = np.ascontiguousarray(x.transpose(0, 3, 1, 2, 4).astype(NBF))
    wqk, wv, wo, eb0, eb1, bco = _host_prep(
        np.asarray(W_qkv, np.float32),
        np.asarray(W_out, np.float32),
        np.asarray(rel_emb, np.float32),
    )
    nc = build_nc()
    nc.finalize()
    in_maps = [
        {
            "x": x[c],
            "w_qk": wqk,
            "w_v": wv,
            "w_o": wo,
            "expb0": eb0,
            "expb1": eb1,
            "bcones": bco,
        }
        for c in range(NCORES)
    ]
    if _bench:
        _install_ntff_hook()
    res = run_bass_kernel_spmd(nc, in_maps, core_ids=list(range(NCORES)), trace=_bench)
    if _bench:
        kernel._last = res
    out = np.concatenate(
        [np.asarray(res.results[c]["out"], np.float32) for c in range(NCORES)], axis=0
    )
    return out.reshape(B, D, WIN, WIN)


# revision 39
# speedup vs baseline: 1.3738x; 1.0177x over previous
"""Windowed multi-head attention with relative-position bias on 8 TRN2 NeuronCores.

Data-parallel over batch: each of the 8 cores processes 4 of the 32 batch
elements end-to-end; weights and the exponentiated bias table are replicated.

v2 layout strategy (per core, b_local=4), derived from trace analysis of v1:
the Activation engine (exp over 12.8M scores/core) was the bottleneck at ~59
G elem/s because each ACTIVATE carried ~425ns of fixed overhead on [128,320]
tiles. This version restructures stage 2 around few, huge activations:

  - S^T psum is laid out as a 4-bank tile (4 heads x 512 i-cols) plus a
    1-bank tile (4 heads x 113 i-cols packed), so softmax runs as TWO
    activations per (batch, head-group, j-tile): [128,2048] + [128,452].
  - The S(jt+1)-after-exp(jt) psum reuse chain is the pipeline clock:
    cycle = matmul(S) + exp = ~3us, with the ACT engine ~100% busy.
  - bias multiply stays on DVE as two big tensor_mul's per j-tile.
  - P@V pairs share psum banks (rows 0:33 / 64:97) -> 3 PV banks per unit;
    psum = 4(S) + 1(Sc1) + 3(PV) = 8 banks exactly.
  - PV outputs are copied psum->sbuf as 3 bf16 casts, then rearranged into
    the head-major O^T layout by Pool-queue DMAs (cheap dispatch).
  - O^T uses a clean (head,dh)-major 256-row layout (den kept separately),
    so W_out needs no repacking and the out-projection is 2 exact K-tiles.
  - normalize-and-cast is fused into one tensor_mul per batch; output is
    written bf16 (halves the output DMA).
  - stage-1 projections of batch b+1 and stage-3 of batch b are emitted
    between attention units as tensor-engine filler.
"""

import numpy as np
import ml_dtypes

import concourse.bass as bass
import concourse.mybir as mybir
import concourse.tile as tile
from concourse import bacc
from concourse.bass_utils import run_bass_kernel_spmd

# problem shape (hardcoded; kernel.py must be self-contained)
B, D, WIN = 32, 256, 25
N = WIN * WIN            # 625 tokens
P = 128
NPAD = 640               # 5 j-tiles of 128
H, DH = 8, 32            # heads x head_dim
NCORES = 8
BL = B // NCORES         # 4 batch elements per core
KT_D = D // P            # 2 contraction tiles over d
NJT = NPAD // P          # 5 j-tiles
C0, C1 = 512, 113        # i-chunks (chunk0 = one psum bank per head)
ICH = [(0, C0), (C0, C1)]
G = DH + 1               # 33: PV output rows per head (32 outputs + den)
NSTR = 632               # padded i-stride (even # of bf16 for DVE 2x slices)

F32 = mybir.dt.float32
BF16 = mybir.dt.bfloat16
EXP = mybir.ActivationFunctionType.Exp
NBF = ml_dtypes.bfloat16
ACT_SPAN_BANKS = True  # one [128,2048] exp per j-tile vs 4 per-bank exps


def _stage1(nc, b, wqk, wv, xall, persist, ps):
    """QKV projections for batch b. Returns (tiles, group-closures): the
    closures each emit one small psum group through the "sc1" bank and can be
    interleaved into an attention unit's pipeline cycles as PE filler."""
    t_qkT = persist.tile([P, 4, NPAD], BF16, tag=f"qkT{b}", name=f"qkT{b}")
    nc.gpsimd.memset(t_qkT[:, 2:4, N:NPAD], 0.0)  # zero k^T j-pad
    t_V = persist.tile([P, NJT, H, G], BF16, tag=f"V{b}", name=f"V{b}")
    nc.gpsimd.memset(t_V[:], 1.0)
    t_q1p = persist.tile([P, 2, 4, C1], BF16, tag=f"q1p{b}", name=f"q1p{b}")
    nc.gpsimd.memset(t_q1p[:], 0.0)
    groups = []

    def qk_c0(mt):
        def emit():
            acc = ps.tile([P, 512], F32, tag="pv", name="acc", bufs=3)
            for kt in range(KT_D):
                nc.tensor.matmul(
                    acc[:],
                    wqk[:, kt, mt * P : (mt + 1) * P],
                    xall[:, b, kt, 0:C0],
                    start=(kt == 0),
                    stop=(kt == KT_D - 1),
                )
            nc.scalar.copy(t_qkT[:, mt, 0:C0], acc[:])
        return emit

    def qk_c1():
        acc1 = ps.tile([P, 4 * C1], F32, tag="sb", name="acc1", bufs=1)
        for mt in range(4):
            for kt in range(KT_D):
                nc.tensor.matmul(
                    acc1[:, mt * C1 : (mt + 1) * C1],
                    wqk[:, kt, mt * P : (mt + 1) * P],
                    xall[:, b, kt, C0:N],
                    start=(kt == 0),
                    stop=(kt == KT_D - 1),
                    skip_group_check=True,
                )
        nc.vector.tensor_copy(
            out=t_qkT[:, :, C0:N],
            in_=acc1[:].rearrange("p (mt c) -> p mt c", c=C1),
        )
        for g in range(2):
            for hq in range(4):
                off = hq * DH
                nc.vector.tensor_copy(
                    out=t_q1p[off : off + DH, g, hq, :],
                    in_=t_qkT[off : off + DH, g, C0:N],
                )

    def v_proj(jt):
        def emit():
            jr = min(P, N - jt * P)
            acc = ps.tile([P, 512], F32, tag="pv", name="accv", bufs=3)
            for kt in range(KT_D):
                nc.tensor.matmul(
                    acc[:jr, :],
                    xall[:, b, kt, jt * P : jt * P + jr],
                    wv[:, kt, :],
                    start=(kt == 0),
                    stop=(kt == KT_D - 1),
                )
            nc.vector.tensor_copy(
                out=t_V[:jr, jt, :, :DH],
                in_=acc[:jr, :D].rearrange("p (h g) -> p h g", g=DH),
            )
        return emit

    # qk_c1 copies q1p for BOTH g from the c1 columns written by qk_c1 itself,
    # so it must run after... it IS the writer of those columns. Order: c0
    # tiles first (they fill cols 0:512 and q1p c0?? no - q1p holds c1 cols),
    # then qk_c1.
    for mt in range(4):
        groups.append(qk_c0(mt))
    groups.append(qk_c1)
    for jt in range(NJT):
        groups.append(v_proj(jt))
    return (t_qkT, t_V, t_q1p), groups


def _pv_emit(nc, b, g, V, pvp, jt, Eo_a, Eo_b):
    """P@V for one j-tile; c0 pairs share banks via disjoint partition rows,
    c1 heads accumulate onto the memset pv2 (no start flags anywhere)."""
    for hq in range(4):
        h = 4 * g + hq
        row = (hq % 2) * 64
        src = Eo_a[:, hq * C0 : (hq + 1) * C0] if hq < 2 else Eo_b[
            :, (hq - 2) * C0 : (hq - 1) * C0
        ]
        nc.tensor.matmul(
            pvp[hq // 2][row : row + G, 0:C0],
            V[b][:, jt, h, :],
            src,
            start=False,
            stop=False,
            tile_position=(0, row),
            skip_group_check=True,
        )
    for hq in range(4):
        h = 4 * g + hq
        row = (hq % 2) * 64
        nc.tensor.matmul(
            pvp[2][row : row + G, (hq // 2) * 128 : (hq // 2) * 128 + C1],
            V[b][:, jt, h, :],
            Eo_b[:, 2 * C0 + hq * C1 : 2 * C0 + (hq + 1) * C1],
            start=False,
            stop=False,
            tile_position=(0, row),
            skip_group_check=True,
        )


def _unit(nc, b, g, qkT, V, q1pad, ebg0, ebg1, OT, den, es, pvs, ps, fillers=None):
    """Attention for (batch b, head-group g): 4 heads, softmax, P@V."""
    import os

    um = int(os.environ.get("UPARTS", "31"))  # 1=Smm 2=acts 4=muls 8=pv 16=out
    # PV accumulators: zeroed up-front, accumulated into WITHOUT start flags
    # (multiple head-groups share each bank; start=True would zero the whole
    # 2KB region under the neighbours).
    pv0 = ps.tile([P, 512], F32, tag="pv", name="pv0", bufs=3)
    pv1 = ps.tile([P, 512], F32, tag="pv", name="pv1", bufs=3)
    pv2 = ps.tile([P, 512], F32, tag="pv", name="pv2", bufs=3)
    nc.vector.memset(pv0[:], 0.0)
    nc.vector.memset(pv1[:], 0.0)
    nc.vector.memset(pv2[:], 0.0)
    pvp = [pv0, pv1, pv2]
    q1p = q1pad[b][:, g]
    # Software pipeline: the act0(jt) -> S0-matmuls(jt+1) psum-reuse chain is
    # the clock (~2.9us/jt). Everything else trails behind it: act1(jt-1)
    # runs BEFORE act0(jt) so the SC1 bank frees early, mul1 runs on the
    # idle GpSimd, and P@V of chunk1 trails TWO j-tiles so its operands are
    # always ready when the in-order PE queue reaches it.
    SB = 2 * C0 + 4 * C1  # 1476: hq2/hq3 c0 + all-heads c1 in one psum tile
    prev = None  # (jt, Eo_a, Eo_b) trailing one step
    for jt in range(NJT):
        s0a = ps.tile([P, 2 * C0], F32, tag="s0a", name="s0a", bufs=1)
        for hq in range(2):
            off = hq * DH
            nc.tensor.matmul(
                s0a[:, hq * C0 : (hq + 1) * C0],
                qkT[b][off : off + DH, 2 + g, jt * P : (jt + 1) * P],
                qkT[b][off : off + DH, g, 0:C0],
                tile_position=(off, 0),
            )
        E0a = es.tile([P, 2 * C0], BF16, tag="E0a", name="E0a", bufs=3)
        nc.scalar.activation(E0a[:], s0a[:], EXP)
        sb = ps.tile([P, 1536], F32, tag="sb", name="sb", bufs=1)
        for hq in range(2, 4):
            off = hq * DH
            nc.tensor.matmul(
                sb[:, (hq - 2) * C0 : (hq - 1) * C0],
                qkT[b][off : off + DH, 2 + g, jt * P : (jt + 1) * P],
                qkT[b][off : off + DH, g, 0:C0],
                tile_position=(off, 0),
            )
        for hq in range(4):
            nc.tensor.matmul(
                sb[:, 2 * C0 + hq * C1 : 2 * C0 + (hq + 1) * C1],
                qkT[b][:, 2 + g, jt * P : (jt + 1) * P],
                q1p[:, hq, :],
                skip_group_check=True,
            )
        E0b = es.tile([P, SB], BF16, tag="E0b", name="E0b", bufs=3)
        nc.scalar.activation(E0b[:], sb[:, :SB], EXP)
        Eo_a = es.tile([P, 2 * C0], BF16, tag="Eo0", name="Eo_a", bufs=3)
        nc.vector.tensor_mul(out=Eo_a[:], in0=E0a[:], in1=ebg0[g][:, jt, :])
        Eo_b = es.tile([P, SB], BF16, tag="Eob", name="Eo_b", bufs=3)
        nc.vector.tensor_mul(out=Eo_b[:], in0=E0b[:], in1=ebg1[g][:, jt, :])
        if prev is not None:
            pjt, pa, pb = prev
            _pv_emit(nc, b, g, V, pvp, pjt, pa, pb)
        prev = (jt, Eo_a, Eo_b)
    pjt, pa, pb = prev
    _pv_emit(nc, b, g, V, pvp, pjt, pa, pb)
    # --- drain PV psum -> bf16 staging -> O^T/den assembly DMAs ---
    pvS = pvs.tile([P, 1280], BF16, tag="pvS", name="pvS", bufs=3)
    nc.vector.tensor_copy(out=pvS[:, 0:512], in_=pv0[:])
    nc.vector.tensor_copy(out=pvS[:, 512:1024], in_=pv1[:])
    nc.vector.tensor_copy(out=pvS[:, 1024:1265], in_=pv2[:, 0:241])
    # heads hq = 2*colblock + partblock; dh rows 0:32 / 64:96, den at 32/96
    for cb in range(2):
        for pb in range(2):
            hq = 2 * cb + pb
            h0 = 32 * g + hq
            nc.sync.dma_start(
                OT[b][hq * DH : (hq + 1) * DH, g, 0:C0],
                pvS[pb * 64 : pb * 64 + DH, cb * 512 : (cb + 1) * 512],
            )
            nc.sync.dma_start(
                den[b][h0 : h0 + 1, 0:C0],
                pvS[pb * 64 + DH : pb * 64 + DH + 1, cb * 512 : (cb + 1) * 512],
            )
            # chunk1: packed at 128-col blocks inside pv2's staging area
            c1b = 1024 + cb * 128
            nc.sync.dma_start(
                OT[b][hq * DH : (hq + 1) * DH, g, C0:N],
                pvS[pb * 64 : pb * 64 + DH, c1b : c1b + C1],
            )
            nc.sync.dma_start(
                den[b][h0 : h0 + 1, C0:N],
                pvS[pb * 64 + DH : pb * 64 + DH + 1, c1b : c1b + C1],
            )


def _pv_matmuls(nc, b, g, V, pvp, eo, jt):
    """P@V for one j-tile; pairs of heads share a psum bank (rows 0/64).
    Accumulates onto the memset zeros — no start/stop flags."""
    Eo0, Eo1 = eo
    for hq in range(4):
        h = 4 * g + hq
        row = (hq % 2) * 64
        nc.tensor.matmul(
            pvp[hq // 2][row : row + G, 0:C0],
            V[b][:, jt, h, :],
            Eo0[:, hq * C0 : (hq + 1) * C0],
            start=False,
            stop=False,
            tile_position=(0, row),
            skip_group_check=True,
        )
    for hq in range(4):
        h = 4 * g + hq
        row = (hq % 2) * 64
        nc.tensor.matmul(
            pvp[2][row : row + G, (hq // 2) * 128 : (hq // 2) * 128 + C1],
            V[b][:, jt, h, :],
            Eo1[:, hq * C1 : (hq + 1) * C1],
            start=False,
            stop=False,
            tile_position=(0, row),
            skip_group_check=True,
        )


def _stage3(nc, b, OT, den, wo, bco, bcs, ys, ps, out_d):
    """Normalize O^T by softmax denominators (broadcast built ON THE PE:
    ones-pattern stationary x rden moving), apply W_out, store bf16."""
    # den rows live at partitions kt*32 + hq so the broadcast matmuls get
    # legal tile positions; pad rows are memset to 1.0 (recip-safe)
    denf = bcs.tile([64, N], F32, tag="denf", name="denf", bufs=2)
    nc.vector.tensor_copy(out=denf[:], in_=den[b][:, :N])
    scr = bcs.tile([64, N], F32, tag="scr", name="scr", bufs=2)
    rden = bcs.tile([64, N], F32, tag="rden", name="rden", bufs=2)
    nc.vector.reciprocal_approx_accurate(out=rden[:], in_=denf[:], scratch=scr[:])
    rdenb = bcs.tile([64, NSTR], BF16, tag="rdenb", name="rdenb", bufs=2)
    nc.vector.tensor_copy(out=rdenb[:, :N], in_=rden[:])
    bcp = {}
    for kt in range(KT_D):
        bcp[kt] = ps.tile([P, 512], F32, tag="pv", name=f"bcp{kt}", bufs=3)
        nc.tensor.matmul(
            bcp[kt][:],
            bco[kt * 32 : kt * 32 + 4, :],
            rdenb[kt * 32 : kt * 32 + 4, 0:C0],
            tile_position=(kt * 32, 0),
        )
    for kt in range(KT_D):
        # separate sc1 allocations: two row-group streams must not share a
        # psum bank concurrently; the WAR chain via the tag serializes them
        bcc = ps.tile([P, 4 * C1], F32, tag="sb", name=f"bcc{kt}", bufs=1)
        nc.tensor.matmul(
            bcc[:, 0:C1],
            bco[kt * 32 : kt * 32 + 4, :],
            rdenb[kt * 32 : kt * 32 + 4, C0:N],
            tile_position=(kt * 32, 0),
            skip_group_check=True,
        )
        nc.vector.tensor_mul(
            out=OT[b][:, kt, 0:C0], in0=OT[b][:, kt, 0:C0], in1=bcp[kt][:]
        )
        nc.vector.tensor_mul(
            out=OT[b][:, kt, C0:N], in0=OT[b][:, kt, C0:N], in1=bcc[:, 0:C1]
        )
    yb = ys.tile([P, KT_D, NSTR], BF16, tag="yb", name="yb", bufs=2)
    for mt in range(KT_D):
        yp = ps.tile([P, 512], F32, tag="pv", name="yp", bufs=3)
        for kt in range(KT_D):
            nc.tensor.matmul(
                yp[:],
                wo[:, kt, mt * P : (mt + 1) * P],
                OT[b][:, kt, 0:C0],
                start=(kt == 0),
                stop=(kt == KT_D - 1),
            )
        nc.scalar.copy(yb[:, mt, 0:C0], yp[:])
    # ragged i-chunk: both mt tiles in the 1-bank "sc1" slot
    yp1 = ps.tile([P, 4 * C1], F32, tag="sb", name="yp1", bufs=1)
    for mt in range(KT_D):
        for kt in range(KT_D):
            nc.tensor.matmul(
                yp1[:, mt * C1 : (mt + 1) * C1],
                wo[:, kt, mt * P : (mt + 1) * P],
                OT[b][:, kt, C0:N],
                start=(kt == 0),
                stop=(kt == KT_D - 1),
                skip_group_check=True,
            )
    nc.vector.tensor_copy(
        out=yb[:, :, C0:N],
        in_=yp1[:, : 2 * C1].rearrange("p (mt c) -> p mt c", c=C1),
    )
    for mt in range(KT_D):
        nc.sync.dma_start(
            out_d[b][mt * P : (mt + 1) * P, :], yb[:, mt, :N]
        )


def build_nc():
    nc = bacc.Bacc()
    x_d = nc.dram_tensor("x", (P, BL, KT_D, N), BF16, kind="ExternalInput")
    wqk_d = nc.dram_tensor("w_qk", (D, 2 * D), BF16, kind="ExternalInput")
    wv_d = nc.dram_tensor("w_v", (D, 2 * D), BF16, kind="ExternalInput")
    wo_d = nc.dram_tensor("w_o", (D, D), BF16, kind="ExternalInput")
    eb0_d = nc.dram_tensor("expb0", (2, NJT, P, 2 * C0), BF16, kind="ExternalInput")
    eb1_d = nc.dram_tensor(
        "expb1", (2, NJT, P, 2 * C0 + 4 * C1), BF16, kind="ExternalInput"
    )
    bco_d = nc.dram_tensor("bcones", (64, P), BF16, kind="ExternalInput")
    out_d = nc.dram_tensor("out", (BL, D, N), BF16, kind="ExternalOutput")

    with tile.TileContext(nc) as tc:
        with (
            tc.tile_pool(name="consts", bufs=1) as consts,
            tc.tile_pool(name="persist", bufs=1) as persist,
            tc.tile_pool(name="ebs", bufs=1) as ebs,
            tc.tile_pool(name="es", bufs=2) as es,
            tc.tile_pool(name="pvs", bufs=2) as pvs,
            tc.tile_pool(name="bcs", bufs=2) as bcs,
            tc.tile_pool(name="ys", bufs=2) as ys,
            tc.tile_pool(name="ps", bufs=1, space="PSUM") as ps,
        ):
            # ---------------- inputs ----------------
            wqk = consts.tile([P, KT_D, 2 * D], BF16)
            nc.sync.dma_start(wqk[:], wqk_d.rearrange("(kt p) m -> p kt m", p=P))
            xall = consts.tile([P, BL, KT_D, N], BF16)
            nc.sync.dma_start(xall[:], x_d[:, :, :, :])
            wv = consts.tile([P, KT_D, 2 * D], BF16)
            nc.sync.dma_start(wv[:], wv_d.rearrange("(kt p) m -> p kt m", p=P))
            wo = consts.tile([P, KT_D, D], BF16)
            nc.sync.dma_start(wo[:], wo_d.rearrange("(kt p) m -> p kt m", p=P))
            bco = consts.tile([64, P], BF16)
            nc.sync.dma_start(bco[:], bco_d[:, :])
            ebg0, ebg1 = {}, {}
            for g in range(2):
                ebg0[g] = ebs.tile(
                    [P, NJT, 2 * C0], BF16, tag=f"eb0g{g}", name=f"eb0g{g}", bufs=1
                )
                nc.sync.dma_start(ebg0[g][:], eb0_d[g].rearrange("jt p c -> p jt c"))
                ebg1[g] = ebs.tile(
                    [P, NJT, 2 * C0 + 4 * C1],
                    BF16, tag=f"eb1g{g}", name=f"eb1g{g}", bufs=1,
                )
                nc.sync.dma_start(ebg1[g][:], eb1_d[g].rearrange("jt p c -> p jt c"))

            # per-batch persistent tiles
            qkT, V, OT, den, q1pad = {}, {}, {}, {}, {}
            for b in range(BL):
                OT[b] = persist.tile(
                    [P, KT_D, NSTR], BF16, tag=f"OT{b}", name=f"OT{b}"
                )
                den[b] = persist.tile(
                    [64, NSTR], BF16, tag=f"den{b}", name=f"den{b}"
                )
                nc.gpsimd.memset(den[b][:], 1.0)

            # stage 1 for b=0 up front; later batches interleave as PE filler
            import os

            lvl = os.environ.get("KBISECT", "full")
            (qkT[0], V[0], q1pad[0]), g0 = _stage1(nc, 0, wqk, wv, xall, persist, ps)
            for grp in g0:
                grp()
            if lvl == "s1":
                pass
            else:
                for b in range(BL):
                    _unit(nc, b, 0, qkT, V, q1pad, ebg0, ebg1, OT, den, es, pvs, ps)
                    if b + 1 < BL:
                        (qkT[b + 1], V[b + 1], q1pad[b + 1]), grps = _stage1(
                            nc, b + 1, wqk, wv, xall, persist, ps
                        )
                        for grp in grps:
                            grp()
                    if b > 0:
                        _stage3(nc, b - 1, OT, den, wo, bco, bcs, ys, ps, out_d)
                    _unit(nc, b, 1, qkT, V, q1pad, ebg0, ebg1, OT, den, es, pvs, ps)
                _stage3(nc, BL - 1, OT, den, wo, bco, bcs, ys, ps, out_d)
    return nc


def _host_prep(W_qkv, W_out, rel_emb):
    scale = DH ** -0.5
    wqk = np.ascontiguousarray(W_qkv[:, : 2 * D]).copy()
    wqk[:, :D] *= scale  # fold q scaling into the weights
    wv = np.zeros((D, 2 * D), np.float32)
    wv[:, :D] = W_qkv[:, 2 * D :]
    wo = np.ascontiguousarray(W_out)  # (h,dh)-major rows match O^T layout
    # relative-position bias -> exp(bias)^T in the packed S-psum layouts
    pos = np.arange(WIN)
    gi, gj = np.meshgrid(pos, pos, indexing="ij")
    grid = np.stack([gi.reshape(-1), gj.reshape(-1)], -1)
    rel = grid[:, None, :] - grid[None, :, :] + (WIN - 1)
    idx = rel[..., 0] * (2 * WIN - 1) + rel[..., 1]  # [i, j]
    ebT = np.exp(rel_emb[idx]).transpose(2, 1, 0)  # -> [h, j, i]
    ebp = np.zeros((H, NPAD, N), np.float32)
    ebp[:, :N, :] = ebT
    e = ebp.reshape(2, 4, NJT, P, N)  # [g, hq, jt, p, i]
    c0p = e[:, :, :, :, :C0].transpose(0, 2, 3, 1, 4)  # [g, jt, p, hq, 512]
    c1p = e[:, :, :, :, C0:].transpose(0, 2, 3, 1, 4)  # [g, jt, p, hq, 113]
    eb0 = np.ascontiguousarray(c0p[:, :, :, :2].reshape(2, NJT, P, 2 * C0))
    eb1 = np.ascontiguousarray(
        np.concatenate(
            [c0p[:, :, :, 2:].reshape(2, NJT, P, 2 * C0),
             c1p.reshape(2, NJT, P, 4 * C1)],
            axis=3,
        )
    )
    bco = np.zeros((64, P), np.float32)
    for kt in range(KT_D):
        for j in range(4):
            bco[kt * 32 + j, j * DH : (j + 1) * DH] = 1.0
    return (
        wqk.astype(NBF),
        wv.astype(NBF),
        wo.astype(NBF),
        eb0.astype(NBF),
        eb1.astype(NBF),
        bco.astype(NBF),
    )


def _install_ntff_hook():
    """This image lacks antenv.axon_hooks; shim it and register the ctypes
    NTFF profiling hook so trace=True yields exec_time_ns. Bench-only."""
    import sys
    import types

    if "antenv.axon_hooks" not in sys.modules:
        mod = types.ModuleType("antenv.axon_hooks")
        mod._hook = None
        mod.set_axon_ntff_profile_hook = lambda h: setattr(mod, "_hook", h)
        mod.get_axon_ntff_profile_hook = lambda: mod._hook
        sys.modules["antenv.axon_hooks"] = mod
    try:
        from trn_agent_boot.trn_boot import _ntff_profile_via_ctypes

        hook = _ntff_profile_via_ctypes("/opt/axon/libaxon_pjrt.so")
        sys.modules["antenv.axon_hooks"].set_axon_ntff_profile_hook(hook)
    except Exception as e:  # degrade to untimed run
        print(f"NTFF hook install failed ({e}); running without trace")


def kernel(x, W_qkv, W_out, rel_emb, _bench=False):
    # pre-transpose per-core x to the sbuf layout so the load DMA is one
    # contiguous 10KB descriptor per partition instead of 1024 row gathers
    x = np.asarray(x, np.float32).reshape(NCORES, BL, KT_D, P, N)
    x = np.ascontiguousarray(x.transpose(0, 3, 1, 2, 4).astype(NBF))
    wqk, wv, wo, eb0, eb1, bco = _host_prep(
        np.asarray(W_qkv, np.float32),
        np.asarray(W_out, np.float32),
        np.asarray(rel_emb, np.float32),
    )
    nc = build_nc()
    nc.finalize()
    in_maps = [
        {
            "x": x[c],
            "w_qk": wqk,
            "w_v": wv,
            "w_o": wo,
            "expb0": eb0,
            "expb1": eb1,
            "bcones": bco,
        }
        for c in range(NCORES)
    ]
    if _bench:
        _install_ntff_hook()
    res = run_bass_kernel_spmd(nc, in_maps, core_ids=list(range(NCORES)), trace=_bench)
    if _bench:
        kernel._last = res
    out = np.concatenate(
        [np.asarray(res.results[c]["out"], np.float32) for c in range(NCORES)], axis=0
    )
    return out.reshape(B, D, WIN, WIN)


# revision 40
# speedup vs baseline: 1.4260x; 1.0380x over previous
"""Windowed multi-head attention with relative-position bias on 8 TRN2 NeuronCores.

Data-parallel over batch: each of the 8 cores processes 4 of the 32 batch
elements end-to-end; weights and the exponentiated bias table are replicated.

v2 layout strategy (per core, b_local=4), derived from trace analysis of v1:
the Activation engine (exp over 12.8M scores/core) was the bottleneck at ~59
G elem/s because each ACTIVATE carried ~425ns of fixed overhead on [128,320]
tiles. This version restructures stage 2 around few, huge activations:

  - S^T psum is laid out as a 4-bank tile (4 heads x 512 i-cols) plus a
    1-bank tile (4 heads x 113 i-cols packed), so softmax runs as TWO
    activations per (batch, head-group, j-tile): [128,2048] + [128,452].
  - The S(jt+1)-after-exp(jt) psum reuse chain is the pipeline clock:
    cycle = matmul(S) + exp = ~3us, with the ACT engine ~100% busy.
  - bias multiply stays on DVE as two big tensor_mul's per j-tile.
  - P@V pairs share psum banks (rows 0:33 / 64:97) -> 3 PV banks per unit;
    psum = 4(S) + 1(Sc1) + 3(PV) = 8 banks exactly.
  - PV outputs are copied psum->sbuf as 3 bf16 casts, then rearranged into
    the head-major O^T layout by Pool-queue DMAs (cheap dispatch).
  - O^T uses a clean (head,dh)-major 256-row layout (den kept separately),
    so W_out needs no repacking and the out-projection is 2 exact K-tiles.
  - normalize-and-cast is fused into one tensor_mul per batch; output is
    written bf16 (halves the output DMA).
  - stage-1 projections of batch b+1 and stage-3 of batch b are emitted
    between attention units as tensor-engine filler.
"""

import numpy as np
import ml_dtypes

import concourse.bass as bass
import concourse.mybir as mybir
import concourse.tile as tile
from concourse import bacc
from concourse.bass_utils import run_bass_kernel_spmd

# problem shape (hardcoded; kernel.py must be self-contained)
B, D, WIN = 32, 256, 25
N = WIN * WIN            # 625 tokens
P = 128
NPAD = 640               # 5 j-tiles of 128
H, DH = 8, 32            # heads x head_dim
NCORES = 8
BL = B // NCORES         # 4 batch elements per core
KT_D = D // P            # 2 contraction tiles over d
NJT = NPAD // P          # 5 j-tiles
C0, C1 = 512, 113        # i-chunks (chunk0 = one psum bank per head)
ICH = [(0, C0), (C0, C1)]
G = DH + 1               # 33: PV output rows per head (32 outputs + den)
NSTR = 632               # padded i-stride (even # of bf16 for DVE 2x slices)

F32 = mybir.dt.float32
BF16 = mybir.dt.bfloat16
EXP = mybir.ActivationFunctionType.Exp
NBF = ml_dtypes.bfloat16
ACT_SPAN_BANKS = True  # one [128,2048] exp per j-tile vs 4 per-bank exps


def _stage1(nc, b, wqk, wv, xall, persist, ps):
    """QKV projections for batch b. Returns (tiles, group-closures): the
    closures each emit one small psum group through the "sc1" bank and can be
    interleaved into an attention unit's pipeline cycles as PE filler."""
    t_qkT = persist.tile([P, 4, NPAD], BF16, tag=f"qkT{b}", name=f"qkT{b}")
    nc.gpsimd.memset(t_qkT[:, 2:4, N:NPAD], 0.0)  # zero k^T j-pad
    t_V = persist.tile([P, NJT, H, G], BF16, tag=f"V{b}", name=f"V{b}")
    nc.gpsimd.memset(t_V[:], 1.0)
    t_q1p = persist.tile([P, 2, 4, C1], BF16, tag=f"q1p{b}", name=f"q1p{b}")
    nc.gpsimd.memset(t_q1p[:], 0.0)
    groups = []

    def qk_c0(mt):
        def emit():
            acc = ps.tile([P, 512], F32, tag="pv", name="acc", bufs=3)
            for kt in range(KT_D):
                nc.tensor.matmul(
                    acc[:],
                    wqk[:, kt, mt * P : (mt + 1) * P],
                    xall[:, b, kt, 0:C0],
                    start=(kt == 0),
                    stop=(kt == KT_D - 1),
                )
            nc.scalar.copy(t_qkT[:, mt, 0:C0], acc[:])
        return emit

    def qk_c1():
        acc1 = ps.tile([P, 4 * C1], F32, tag="sb", name="acc1", bufs=1)
        for mt in range(4):
            for kt in range(KT_D):
                nc.tensor.matmul(
                    acc1[:, mt * C1 : (mt + 1) * C1],
                    wqk[:, kt, mt * P : (mt + 1) * P],
                    xall[:, b, kt, C0:N],
                    start=(kt == 0),
                    stop=(kt == KT_D - 1),
                    skip_group_check=True,
                )
        nc.vector.tensor_copy(
            out=t_qkT[:, :, C0:N],
            in_=acc1[:].rearrange("p (mt c) -> p mt c", c=C1),
        )
        for g in range(2):
            for hq in range(4):
                off = hq * DH
                nc.vector.tensor_copy(
                    out=t_q1p[off : off + DH, g, hq, :],
                    in_=t_qkT[off : off + DH, g, C0:N],
                )

    def v_proj(jt):
        def emit():
            jr = min(P, N - jt * P)
            acc = ps.tile([P, 512], F32, tag="pv", name="accv", bufs=3)
            for kt in range(KT_D):
                nc.tensor.matmul(
                    acc[:jr, :],
                    xall[:, b, kt, jt * P : jt * P + jr],
                    wv[:, kt, :],
                    start=(kt == 0),
                    stop=(kt == KT_D - 1),
                )
            nc.vector.tensor_copy(
                out=t_V[:jr, jt, :, :DH],
                in_=acc[:jr, :D].rearrange("p (h g) -> p h g", g=DH),
            )
        return emit

    # qk_c1 copies q1p for BOTH g from the c1 columns written by qk_c1 itself,
    # so it must run after... it IS the writer of those columns. Order: c0
    # tiles first (they fill cols 0:512 and q1p c0?? no - q1p holds c1 cols),
    # then qk_c1.
    for mt in range(4):
        groups.append(qk_c0(mt))
    groups.append(qk_c1)
    for jt in range(NJT):
        groups.append(v_proj(jt))
    return (t_qkT, t_V, t_q1p), groups


def _pv_emit(nc, b, g, V, pvp, jt, Eo_a, Eo_b):
    """P@V for one j-tile; c0 pairs share banks via disjoint partition rows,
    c1 heads accumulate onto the memset pv2 (no start flags anywhere)."""
    for hq in range(4):
        h = 4 * g + hq
        row = (hq % 2) * 64
        src = Eo_a[:, hq * C0 : (hq + 1) * C0] if hq < 2 else Eo_b[
            :, (hq - 2) * C0 : (hq - 1) * C0
        ]
        nc.tensor.matmul(
            pvp[hq // 2][row : row + G, 0:C0],
            V[b][:, jt, h, :],
            src,
            start=False,
            stop=False,
            tile_position=(0, row),
            skip_group_check=True,
        )
    for hq in range(4):
        h = 4 * g + hq
        row = (hq % 2) * 64
        nc.tensor.matmul(
            pvp[2][row : row + G, (hq // 2) * 128 : (hq // 2) * 128 + C1],
            V[b][:, jt, h, :],
            Eo_b[:, 2 * C0 + hq * C1 : 2 * C0 + (hq + 1) * C1],
            start=False,
            stop=False,
            tile_position=(0, row),
            skip_group_check=True,
        )


def _unit(nc, b, g, qkT, V, q1pad, ebg0, ebg1, OT, den, es, pvs, ps, fillers=None):
    """Attention for (batch b, head-group g): 4 heads, softmax, P@V."""
    import os

    um = int(os.environ.get("UPARTS", "31"))  # 1=Smm 2=acts 4=muls 8=pv 16=out
    # PV accumulators: zeroed up-front, accumulated into WITHOUT start flags
    # (multiple head-groups share each bank; start=True would zero the whole
    # 2KB region under the neighbours).
    pv0 = ps.tile([P, 512], F32, tag="pv", name="pv0", bufs=3)
    pv1 = ps.tile([P, 512], F32, tag="pv", name="pv1", bufs=3)
    pv2 = ps.tile([P, 512], F32, tag="pv", name="pv2", bufs=3)
    nc.vector.memset(pv0[:], 0.0)
    nc.vector.memset(pv1[:], 0.0)
    nc.vector.memset(pv2[:], 0.0)
    pvp = [pv0, pv1, pv2]
    q1p = q1pad[b][:, g]
    # Software pipeline: the act0(jt) -> S0-matmuls(jt+1) psum-reuse chain is
    # the clock (~2.9us/jt). Everything else trails behind it: act1(jt-1)
    # runs BEFORE act0(jt) so the SC1 bank frees early, mul1 runs on the
    # idle GpSimd, and P@V of chunk1 trails TWO j-tiles so its operands are
    # always ready when the in-order PE queue reaches it.
    SB = 2 * C0 + 4 * C1  # 1476: hq2/hq3 c0 + all-heads c1 in one psum tile
    prev = None  # (jt, Eo_a, Eo_b) trailing one step
    for jt in range(NJT):
        s0a = ps.tile([P, 2 * C0], F32, tag="s0a", name="s0a", bufs=1)
        for hq in range(2):
            off = hq * DH
            nc.tensor.matmul(
                s0a[:, hq * C0 : (hq + 1) * C0],
                qkT[b][off : off + DH, 2 + g, jt * P : (jt + 1) * P],
                qkT[b][off : off + DH, g, 0:C0],
                tile_position=(off, 0),
            )
        E0a = es.tile([P, 2 * C0], BF16, tag="E0a", name="E0a", bufs=3)
        nc.scalar.activation(E0a[:], s0a[:], EXP)
        sb = ps.tile([P, 1536], F32, tag="sb", name="sb", bufs=1)
        for hq in range(2, 4):
            off = hq * DH
            nc.tensor.matmul(
                sb[:, (hq - 2) * C0 : (hq - 1) * C0],
                qkT[b][off : off + DH, 2 + g, jt * P : (jt + 1) * P],
                qkT[b][off : off + DH, g, 0:C0],
                tile_position=(off, 0),
            )
        for hq in range(4):
            nc.tensor.matmul(
                sb[:, 2 * C0 + hq * C1 : 2 * C0 + (hq + 1) * C1],
                qkT[b][:, 2 + g, jt * P : (jt + 1) * P],
                q1p[:, hq, :],
                skip_group_check=True,
            )
        E0b = es.tile([P, SB], BF16, tag="E0b", name="E0b", bufs=3)
        nc.scalar.activation(E0b[:], sb[:, :SB], EXP)
        Eo_a = es.tile([P, 2 * C0], BF16, tag="Eo0", name="Eo_a", bufs=3)
        nc.vector.tensor_mul(out=Eo_a[:], in0=E0a[:], in1=ebg0[g][:, jt, :])
        Eo_b = es.tile([P, SB], BF16, tag="Eob", name="Eo_b", bufs=3)
        nc.vector.tensor_mul(out=Eo_b[:], in0=E0b[:], in1=ebg1[g][:, jt, :])
        if prev is not None:
            pjt, pa, pb = prev
            _pv_emit(nc, b, g, V, pvp, pjt, pa, pb)
        prev = (jt, Eo_a, Eo_b)
    pjt, pa, pb = prev
    _pv_emit(nc, b, g, V, pvp, pjt, pa, pb)
    # --- drain PV psum -> bf16 staging -> O^T/den assembly DMAs ---
    pvS = pvs.tile([P, 1280], BF16, tag="pvS", name="pvS", bufs=3)
    nc.vector.tensor_copy(out=pvS[:, 0:512], in_=pv0[:])
    nc.vector.tensor_copy(out=pvS[:, 512:1024], in_=pv1[:])
    nc.vector.tensor_copy(out=pvS[:, 1024:1265], in_=pv2[:, 0:241])
    # heads hq = 2*colblock + partblock; dh rows 0:32 / 64:96, den at 32/96
    for cb in range(2):
        for pb in range(2):
            hq = 2 * cb + pb
            h0 = 32 * g + hq
            nc.sync.dma_start(
                OT[b][hq * DH : (hq + 1) * DH, g, 0:C0],
                pvS[pb * 64 : pb * 64 + DH, cb * 512 : (cb + 1) * 512],
            )
            nc.gpsimd.dma_start(
                den[b][h0 : h0 + 1, 0:C0],
                pvS[pb * 64 + DH : pb * 64 + DH + 1, cb * 512 : (cb + 1) * 512],
            )
            # chunk1: packed at 128-col blocks inside pv2's staging area
            c1b = 1024 + cb * 128
            nc.gpsimd.dma_start(
                OT[b][hq * DH : (hq + 1) * DH, g, C0:N],
                pvS[pb * 64 : pb * 64 + DH, c1b : c1b + C1],
            )
            nc.gpsimd.dma_start(
                den[b][h0 : h0 + 1, C0:N],
                pvS[pb * 64 + DH : pb * 64 + DH + 1, c1b : c1b + C1],
            )


def _pv_matmuls(nc, b, g, V, pvp, eo, jt):
    """P@V for one j-tile; pairs of heads share a psum bank (rows 0/64).
    Accumulates onto the memset zeros — no start/stop flags."""
    Eo0, Eo1 = eo
    for hq in range(4):
        h = 4 * g + hq
        row = (hq % 2) * 64
        nc.tensor.matmul(
            pvp[hq // 2][row : row + G, 0:C0],
            V[b][:, jt, h, :],
            Eo0[:, hq * C0 : (hq + 1) * C0],
            start=False,
            stop=False,
            tile_position=(0, row),
            skip_group_check=True,
        )
    for hq in range(4):
        h = 4 * g + hq
        row = (hq % 2) * 64
        nc.tensor.matmul(
            pvp[2][row : row + G, (hq // 2) * 128 : (hq // 2) * 128 + C1],
            V[b][:, jt, h, :],
            Eo1[:, hq * C1 : (hq + 1) * C1],
            start=False,
            stop=False,
            tile_position=(0, row),
            skip_group_check=True,
        )


def _stage3(nc, b, OT, den, wo, bco, bcs, ys, ps, out_d):
    """Normalize O^T by softmax denominators (broadcast built ON THE PE:
    ones-pattern stationary x rden moving), apply W_out, store bf16."""
    # den rows live at partitions kt*32 + hq so the broadcast matmuls get
    # legal tile positions; pad rows are memset to 1.0 (recip-safe)
    denf = bcs.tile([64, N], F32, tag="denf", name="denf", bufs=2)
    nc.vector.tensor_copy(out=denf[:], in_=den[b][:, :N])
    scr = bcs.tile([64, N], F32, tag="scr", name="scr", bufs=2)
    rden = bcs.tile([64, N], F32, tag="rden", name="rden", bufs=2)
    nc.vector.reciprocal_approx_accurate(out=rden[:], in_=denf[:], scratch=scr[:])
    rdenb = bcs.tile([64, NSTR], BF16, tag="rdenb", name="rdenb", bufs=2)
    nc.vector.tensor_copy(out=rdenb[:, :N], in_=rden[:])
    bcp = {}
    for kt in range(KT_D):
        bcp[kt] = ps.tile([P, 512], F32, tag="pv", name=f"bcp{kt}", bufs=3)
        nc.tensor.matmul(
            bcp[kt][:],
            bco[kt * 32 : kt * 32 + 4, :],
            rdenb[kt * 32 : kt * 32 + 4, 0:C0],
            tile_position=(kt * 32, 0),
        )
    for kt in range(KT_D):
        # separate sc1 allocations: two row-group streams must not share a
        # psum bank concurrently; the WAR chain via the tag serializes them
        bcc = ps.tile([P, 4 * C1], F32, tag="sb", name=f"bcc{kt}", bufs=1)
        nc.tensor.matmul(
            bcc[:, 0:C1],
            bco[kt * 32 : kt * 32 + 4, :],
            rdenb[kt * 32 : kt * 32 + 4, C0:N],
            tile_position=(kt * 32, 0),
            skip_group_check=True,
        )
        nc.vector.tensor_mul(
            out=OT[b][:, kt, 0:C0], in0=OT[b][:, kt, 0:C0], in1=bcp[kt][:]
        )
        nc.vector.tensor_mul(
            out=OT[b][:, kt, C0:N], in0=OT[b][:, kt, C0:N], in1=bcc[:, 0:C1]
        )
    yb = ys.tile([P, KT_D, NSTR], BF16, tag="yb", name="yb", bufs=2)
    for mt in range(KT_D):
        yp = ps.tile([P, 512], F32, tag="pv", name="yp", bufs=3)
        for kt in range(KT_D):
            nc.tensor.matmul(
                yp[:],
                wo[:, kt, mt * P : (mt + 1) * P],
                OT[b][:, kt, 0:C0],
                start=(kt == 0),
                stop=(kt == KT_D - 1),
            )
        nc.scalar.copy(yb[:, mt, 0:C0], yp[:])
    # ragged i-chunk: both mt tiles in the 1-bank "sc1" slot
    yp1 = ps.tile([P, 4 * C1], F32, tag="sb", name="yp1", bufs=1)
    for mt in range(KT_D):
        for kt in range(KT_D):
            nc.tensor.matmul(
                yp1[:, mt * C1 : (mt + 1) * C1],
                wo[:, kt, mt * P : (mt + 1) * P],
                OT[b][:, kt, C0:N],
                start=(kt == 0),
                stop=(kt == KT_D - 1),
                skip_group_check=True,
            )
    nc.vector.tensor_copy(
        out=yb[:, :, C0:N],
        in_=yp1[:, : 2 * C1].rearrange("p (mt c) -> p mt c", c=C1),
    )
    for mt in range(KT_D):
        nc.sync.dma_start(
            out_d[b][mt * P : (mt + 1) * P, :], yb[:, mt, :N]
        )


def build_nc():
    nc = bacc.Bacc()
    x_d = nc.dram_tensor("x", (P, BL, KT_D, N), BF16, kind="ExternalInput")
    wqk_d = nc.dram_tensor("w_qk", (D, 2 * D), BF16, kind="ExternalInput")
    wv_d = nc.dram_tensor("w_v", (D, 2 * D), BF16, kind="ExternalInput")
    wo_d = nc.dram_tensor("w_o", (D, D), BF16, kind="ExternalInput")
    eb0_d = nc.dram_tensor("expb0", (2, NJT, P, 2 * C0), BF16, kind="ExternalInput")
    eb1_d = nc.dram_tensor(
        "expb1", (2, NJT, P, 2 * C0 + 4 * C1), BF16, kind="ExternalInput"
    )
    bco_d = nc.dram_tensor("bcones", (64, P), BF16, kind="ExternalInput")
    out_d = nc.dram_tensor("out", (BL, D, N), BF16, kind="ExternalOutput")

    with tile.TileContext(nc) as tc:
        with (
            tc.tile_pool(name="consts", bufs=1) as consts,
            tc.tile_pool(name="persist", bufs=1) as persist,
            tc.tile_pool(name="ebs", bufs=1) as ebs,
            tc.tile_pool(name="es", bufs=2) as es,
            tc.tile_pool(name="pvs", bufs=2) as pvs,
            tc.tile_pool(name="bcs", bufs=2) as bcs,
            tc.tile_pool(name="ys", bufs=2) as ys,
            tc.tile_pool(name="ps", bufs=1, space="PSUM") as ps,
        ):
            # ---------------- inputs ----------------
            wqk = consts.tile([P, KT_D, 2 * D], BF16)
            nc.sync.dma_start(wqk[:], wqk_d.rearrange("(kt p) m -> p kt m", p=P))
            xall = consts.tile([P, BL, KT_D, N], BF16)
            nc.sync.dma_start(xall[:], x_d[:, :, :, :])
            wv = consts.tile([P, KT_D, 2 * D], BF16)
            nc.sync.dma_start(wv[:], wv_d.rearrange("(kt p) m -> p kt m", p=P))
            wo = consts.tile([P, KT_D, D], BF16)
            nc.sync.dma_start(wo[:], wo_d.rearrange("(kt p) m -> p kt m", p=P))
            bco = consts.tile([64, P], BF16)
            nc.sync.dma_start(bco[:], bco_d[:, :])
            ebg0, ebg1 = {}, {}
            for g in range(2):
                ebg0[g] = ebs.tile(
                    [P, NJT, 2 * C0], BF16, tag=f"eb0g{g}", name=f"eb0g{g}", bufs=1
                )
                nc.sync.dma_start(ebg0[g][:], eb0_d[g].rearrange("jt p c -> p jt c"))
                ebg1[g] = ebs.tile(
                    [P, NJT, 2 * C0 + 4 * C1],
                    BF16, tag=f"eb1g{g}", name=f"eb1g{g}", bufs=1,
                )
                nc.sync.dma_start(ebg1[g][:], eb1_d[g].rearrange("jt p c -> p jt c"))

            # per-batch persistent tiles
            qkT, V, OT, den, q1pad = {}, {}, {}, {}, {}
            for b in range(BL):
                OT[b] = persist.tile(
                    [P, KT_D, NSTR], BF16, tag=f"OT{b}", name=f"OT{b}"
                )
                den[b] = persist.tile(
                    [64, NSTR], BF16, tag=f"den{b}", name=f"den{b}"
                )
                nc.gpsimd.memset(den[b][:], 1.0)

            # stage 1 for b=0 up front; later batches interleave as PE filler
            import os

            lvl = os.environ.get("KBISECT", "full")
            (qkT[0], V[0], q1pad[0]), g0 = _stage1(nc, 0, wqk, wv, xall, persist, ps)
            for grp in g0:
                grp()
            if lvl == "s1":
                pass
            else:
                for b in range(BL):
                    _unit(nc, b, 0, qkT, V, q1pad, ebg0, ebg1, OT, den, es, pvs, ps)
                    if b + 1 < BL:
                        (qkT[b + 1], V[b + 1], q1pad[b + 1]), grps = _stage1(
                            nc, b + 1, wqk, wv, xall, persist, ps
                        )
                        for grp in grps:
                            grp()
                    if b > 0:
                        _stage3(nc, b - 1, OT, den, wo, bco, bcs, ys, ps, out_d)
                    _unit(nc, b, 1, qkT, V, q1pad, ebg0, ebg1, OT, den, es, pvs, ps)
                _stage3(nc, BL - 1, OT, den, wo, bco, bcs, ys, ps, out_d)
    return nc


def _host_prep(W_qkv, W_out, rel_emb):
    scale = DH ** -0.5
    wqk = np.ascontiguousarray(W_qkv[:, : 2 * D]).copy()
    wqk[:, :D] *= scale  # fold q scaling into the weights
    wv = np.zeros((D, 2 * D), np.float32)
    wv[:, :D] = W_qkv[:, 2 * D :]
    wo = np.ascontiguousarray(W_out)  # (h,dh)-major rows match O^T layout
    # relative-position bias -> exp(bias)^T in the packed S-psum layouts
    pos = np.arange(WIN)
    gi, gj = np.meshgrid(pos, pos, indexing="ij")
    grid = np.stack([gi.reshape(-1), gj.reshape(-1)], -1)
    rel = grid[:, None, :] - grid[None, :, :] + (WIN - 1)
    idx = rel[..., 0] * (2 * WIN - 1) + rel[..., 1]  # [i, j]
    ebT = np.exp(rel_emb[idx]).transpose(2, 1, 0)  # -> [h, j, i]
    ebp = np.zeros((H, NPAD, N), np.float32)
    ebp[:, :N, :] = ebT
    e = ebp.reshape(2, 4, NJT, P, N)  # [g, hq, jt, p, i]
    c0p = e[:, :, :, :, :C0].transpose(0, 2, 3, 1, 4)  # [g, jt, p, hq, 512]
    c1p = e[:, :, :, :, C0:].transpose(0, 2, 3, 1, 4)  # [g, jt, p, hq, 113]
    eb0 = np.ascontiguousarray(c0p[:, :, :, :2].reshape(2, NJT, P, 2 * C0))
    eb1 = np.ascontiguousarray(
        np.concatenate(
            [c0p[:, :, :, 2:].reshape(2, NJT, P, 2 * C0),
             c1p.reshape(2, NJT, P, 4 * C1)],
            axis=3,
        )
    )
    bco = np.zeros((64, P), np.float32)
    for kt in range(KT_D):
        for j in range(4):
            bco[kt * 32 + j, j * DH : (j + 1) * DH] = 1.0
    return (
        wqk.astype(NBF),
        wv.astype(NBF),
        wo.astype(NBF),
        eb0.astype(NBF),
        eb1.astype(NBF),
        bco.astype(NBF),
    )


def _install_ntff_hook():
    """This image lacks antenv.axon_hooks; shim it and register the ctypes
    NTFF profiling hook so trace=True yields exec_time_ns. Bench-only."""
    import sys
    import types

    if "antenv.axon_hooks" not in sys.modules:
        mod = types.ModuleType("antenv.axon_hooks")
        mod._hook = None
        mod.set_axon_ntff_profile_hook = lambda h: setattr(mod, "_hook", h)
        mod.get_axon_ntff_profile_hook = lambda: mod._hook
        sys.modules["antenv.axon_hooks"] = mod
    try:
        from trn_agent_boot.trn_boot import _ntff_profile_via_ctypes

        hook = _ntff_profile_via_ctypes("/opt/axon/libaxon_pjrt.so")
        sys.modules["antenv.axon_hooks"].set_axon_ntff_profile_hook(hook)
    except Exception as e:  # degrade to untimed run
        print(f"NTFF hook install failed ({e}); running without trace")


def kernel(x, W_qkv, W_out, rel_emb, _bench=False):
    # pre-transpose per-core x to the sbuf layout so the load DMA is one
    # contiguous 10KB descriptor per partition instead of 1024 row gathers
    x = np.asarray(x, np.float32).reshape(NCORES, BL, KT_D, P, N)
    x = np.ascontiguousarray(x.transpose(0, 3, 1, 2, 4).astype(NBF))
    wqk, wv, wo, eb0, eb1, bco = _host_prep(
        np.asarray(W_qkv, np.float32),
        np.asarray(W_out, np.float32),
        np.asarray(rel_emb, np.float32),
    )
    nc = build_nc()
    nc.finalize()
    in_maps = [
        {
            "x": x[c],
            "w_qk": wqk,
            "w_v": wv,
            "w_o": wo,
            "expb0": eb0,
            "expb1": eb1,
            "bcones": bco,
        }
        for c in range(NCORES)
    ]
    if _bench:
        _install_ntff_hook()
    res = run_bass_kernel_spmd(nc, in_maps, core_ids=list(range(NCORES)), trace=_bench)
    if _bench:
        kernel._last = res
    out = np.concatenate(
        [np.asarray(res.results[c]["out"], np.float32) for c in range(NCORES)], axis=0
    )
    return out.reshape(B, D, WIN, WIN)
